# revision 1
# baseline (speedup 1.0000x reference)
"""Chamfer-loss kernel for 8 Trainium2 NeuronCores.

Problem (hardcoded shapes): B=2, N=M=8192, points in R^3.
  d2[b,n,m] = |p_n|^2 + |r_m|^2 - 2 p_n.r_m   (clamped at 0)
  closest[b,m] = argmin_n d2           -> gathers + L1 means
  chamfer     = mean_b(mean_n min_m d2 + mean_m min_n d2)

Sharding: core = (b, m-chunk of 2048).  Each core computes its
[2048 x 8192] slab of d2 on the fly with a split-fp16 augmented matmul
(K=18 rows carry all hi/lo cross products of (-2*r) x p plus 3-way
splits of |p|^2 and |r|^2, giving ~1e-6 absolute d2 accuracy at the
fp16 1-column/cycle PE rate).  ScalarE drains PSUM to fp16 SBUF with a
fused relu; VectorE then does all min-reductions as fp16 folds at the
2x DVE rate:
  - winm:   per-window partial mins over n (window=8, kept at width 2)
  - rowacc: elementwise running min over the 16 m-blocks  [128, 8192]
The host then: takes the top-2 windows per m from the fp16 winmins and
recomputes those distances exactly in fp32 (=> exact colmin + argmin,
immune to fp16 rounding ties), folds rowacc across partitions/cores
(=> rowmin), and applies the gathers and means.
"""

import numpy as np

B, N, M = 2, 8192, 8192
NCORES = 8
CHUNKS = 4           # m-chunks per batch (cores per batch)
MC = M // CHUNKS     # 2048 ref points per core
NB = MC // 128       # 16 m-blocks of 128 partitions
GFREE = 2048         # psum group free size (4 banks; x2 bufs fills PSUM)
GROUPS = N // GFREE  # psum groups per m-block
QN = GFREE // 512    # matmuls per group
WIN = 8              # window width for argmin recovery
NWIN = N // WIN      # windows per m-row

# matmul input mode: "f32" (simple, PE-slow) or "f16x2" (split fp16, PE-fast)
MM_MODE = "f16x2"
# reduction mode: "f32" (exact, DVE-heavy) or "f16" (ACT drain + fp16 folds)
RED_MODE = "f16"
# ablation for perf debugging: set of stage names to skip
# ("winfold", "rowfold", "drain")
ABLATE = set()
# matmul moving free-dim per instruction (<=512)
MM_FREE = 512
# scheduling knobs
DR_BUFS = 3          # drained-tile double/triple buffering
L1_PER_GROUP = False  # fused whole-beta winfold L1 measured faster
LOOP_HINTS = True    # branch-prefetch hints on the timing loop

_cache = {}


def _split_f16(x):
    """x (f32) -> (hi, lo) fp16 with hi+lo ~= x to ~2^-23 rel."""
    hi = x.astype(np.float16)
    lo = (x - hi.astype(np.float32)).astype(np.float16)
    return hi, lo


def _split3_f16(x):
    hi = x.astype(np.float16)
    r = x - hi.astype(np.float32)
    mid = r.astype(np.float16)
    lo = (r - mid.astype(np.float32)).astype(np.float16)
    return hi, mid, lo


def _build_program(loop_iters=None):
    import contextlib
    import concourse.bacc as bacc
    import concourse.mybir as mybir
    from concourse.tile import TileContext

    f32 = mybir.dt.float32
    f16 = mybir.dt.float16
    mmdt = f32 if MM_MODE == "f32" else f16
    K = 5 if MM_MODE == "f32" else 18

    red16 = RED_MODE == "f16"
    acc_dt = f16 if red16 else f32
    wout = 2 * NWIN if red16 else NWIN  # fp16 path keeps window width 2

    nc = bacc.Bacc(None, target_bir_lowering=False)
    pm = nc.dram_tensor("pm", [K, N], mmdt, kind="ExternalInput")
    rs = nc.dram_tensor("rs", [K, MC], mmdt, kind="ExternalInput")
    winm_out = nc.dram_tensor("winm", [128, NB * wout], acc_dt,
                              kind="ExternalOutput")
    rowacc_out = nc.dram_tensor("rowacc", [128, N], acc_dt,
                                kind="ExternalOutput")

    with TileContext(nc) as tc:
        with tc.tile_pool(name="sb", bufs=1) as sb, \
             tc.tile_pool(name="winp", bufs=2) as winp, \
             tc.tile_pool(name="drp", bufs=2) as drp, \
             tc.tile_pool(name="ps", bufs=2, space="PSUM") as psp:
            hints = ((mybir.EngineType.PE, mybir.EngineType.DVE,
                      mybir.EngineType.Activation, mybir.EngineType.SP)
                     if LOOP_HINTS else ())
            pm_sb = sb.tile([K, N], mmdt)
            rs_sb = sb.tile([K, MC], mmdt)
            nc.sync.dma_start(out=pm_sb[:], in_=pm[:])
            nc.sync.dma_start(out=rs_sb[:], in_=rs[:])
            loop = (tc.For_i(0, loop_iters, 1, hint_engines=hints)
                    if loop_iters else contextlib.nullcontext())
            with loop:
                rowacc = sb.tile([128, N], acc_dt)
                if not red16:
                    nc.vector.memset(rowacc[:], 3.0e38)

                for beta in range(NB):
                    winm_b = winp.tile([128, wout], acc_dt)
                    lhs = rs_sb[:, beta * 128:(beta + 1) * 128]
                    dr = (drp.tile([128, N], f16, name="dr", tag="dr",
                                   bufs=DR_BUFS)
                          if red16 else None)
                    l1b = None
                    if red16 and L1_PER_GROUP and "winfold" not in ABLATE:
                        l1b = drp.tile([128, N // 2], f16, name="l1b",
                                       tag="l1b")
                    for g in range(GROUPS):
                        gsl = slice(g * GFREE, (g + 1) * GFREE)
                        ps = psp.tile([128, GFREE], f32)
                        for q in range(GFREE // MM_FREE):
                            col0 = g * GFREE + q * MM_FREE
                            nc.tensor.matmul(
                                ps[:, q * MM_FREE:(q + 1) * MM_FREE],
                                lhs,
                                pm_sb[:, col0:col0 + MM_FREE],
                                start=True, stop=True,
                            )
                        if not red16:
                            # exact f32 path: both reductions straight from PSUM
                            nwin_g = GFREE // WIN
                            nc.vector.tensor_reduce(
                                winm_b[:, g * nwin_g:(g + 1) * nwin_g],
                                ps[:].rearrange("p (w i) -> p w i", i=WIN),
                                axis=mybir.AxisListType.X,
                                op=mybir.AluOpType.min,
                            )
                            nc.vector.tensor_tensor(
                                out=rowacc[:, gsl], in0=ps[:],
                                in1=rowacc[:, gsl], op=mybir.AluOpType.min,
                            )
                        else:
                            # drain with fused relu to fp16 on ScalarE,
                            # then fp16 folds on DVE at 2x
                            if "drain" not in ABLATE:
                                nc.scalar.activation(
                                    out=dr[:, gsl], in_=ps[:],
                                    func=mybir.ActivationFunctionType.Relu,
                                )
                            elif not {"rowfold", "winfold"} <= ABLATE:
                                nc.vector.memset(dr[:, gsl], 1.0)
                            if l1b is not None:
                                ngw = GFREE // WIN
                                half = WIN // 2
                                cg = dr[:, gsl].rearrange(
                                    "p (w i) -> p w i", i=WIN)
                                og = l1b[:, g * GFREE // 2:
                                         (g + 1) * GFREE // 2].rearrange(
                                    "p (w i) -> p w i", i=half)
                                nc.vector.tensor_tensor(
                                    out=og, in0=cg[:, :, 0:half],
                                    in1=cg[:, :, half:WIN],
                                    op=mybir.AluOpType.min,
                                )
                    if red16 and "rowfold" not in ABLATE:
                        # running elementwise min over m-blocks, whole beta row
                        if beta == 0:
                            nc.vector.tensor_copy(out=rowacc[:], in_=dr[:])
                        else:
                            nc.vector.tensor_tensor(
                                out=rowacc[:], in0=dr[:], in1=rowacc[:],
                                op=mybir.AluOpType.min,
                            )
                    if red16 and "winfold" in ABLATE:
                        nc.gpsimd.memset(winm_b[:], 1.0)
                    if red16 and "winfold" not in ABLATE:
                        # window-min fold tree over n, ending at width 2
                        # (host takes the final pairwise min; every DVE fold
                        # stays at the 2x fp16 rate)
                        nw = NWIN
                        if l1b is not None:
                            cur = l1b[:].rearrange("p (w i) -> p w i",
                                                   i=WIN // 2)
                            width = WIN // 2
                        else:
                            cur = dr[:].rearrange("p (w i) -> p w i", i=WIN)
                            width = WIN
                        while width > 2:
                            half = width // 2
                            if half == 2:
                                nxt = winm_b[:].rearrange("p (w i) -> p w i", i=2)
                            else:
                                nxt_t = drp.tile([128, nw * half], f16,
                                                 tag="fold%d" % half)
                                nxt = nxt_t[:].rearrange("p (w i) -> p w i", i=half)
                            nc.vector.tensor_tensor(
                                out=nxt, in0=cur[:, :, 0:half],
                                in1=cur[:, :, half:width],
                                op=mybir.AluOpType.min,
                            )
                            cur, width = nxt, half
                    nc.sync.dma_start(
                        out=winm_out[:, beta * wout:(beta + 1) * wout],
                        in_=winm_b[:],
                    )
                nc.sync.dma_start(out=rowacc_out[:], in_=rowacc[:])
    nc.compile()
    return nc


def _make_in_maps(pp, rp, pn2, rm2):
    """Build per-core augmented moving/stationary operands."""
    in_maps = []
    for cid in range(NCORES):
        b, c = cid // CHUNKS, cid % CHUNKS
        sl = slice(c * MC, (c + 1) * MC)
        if MM_MODE == "f32":
            pmv = np.empty((5, N), np.float32)
            pmv[0:3] = pp[b].T
            pmv[3] = pn2[b]
            pmv[4] = 1.0
            rsv = np.empty((5, MC), np.float32)
            rsv[0:3] = -2.0 * rp[b, sl].T
            rsv[3] = 1.0
            rsv[4] = rm2[b, sl]
        else:
            # split fp16: cross terms need all 4 hi/lo pairs per dim,
            # norms need 3-way splits against constant-1 rows.
            ph, pl = _split_f16(pp[b].T)            # [3, N] each
            rh, rl = _split_f16(rp[b, sl].T)        # [3, MC]
            n2h, n2m, n2l = _split3_f16(pn2[b])     # [N]
            r2h, r2m, r2l = _split3_f16(rm2[b, sl])  # [MC]
            pmv = np.zeros((18, N), np.float16)
            rsv = np.zeros((18, MC), np.float16)
            # rows 0..11: cross terms, 4 pairs per dim d:
            #   (-2*rh)*ph, (-2*rh)*pl, (-2*rl)*ph, (-2*rl)*pl
            for d in range(3):
                r0 = 4 * d
                pmv[r0 + 0] = ph[d]
                pmv[r0 + 1] = pl[d]
                pmv[r0 + 2] = ph[d]
                pmv[r0 + 3] = pl[d]
                rsv[r0 + 0] = -2.0 * rh[d].astype(np.float32)
                rsv[r0 + 1] = -2.0 * rh[d].astype(np.float32)
                rsv[r0 + 2] = -2.0 * rl[d].astype(np.float32)
                rsv[r0 + 3] = -2.0 * rl[d].astype(np.float32)
            # rows 12..14: |p|^2 split x stationary ones
            pmv[12], pmv[13], pmv[14] = n2h, n2m, n2l
            rsv[12:15] = 1.0
            # rows 15..17: |r|^2 split x moving ones
            pmv[15:18] = 1.0
            rsv[15], rsv[16], rsv[17] = r2h, r2m, r2l
        in_maps.append({"pm": pmv, "rs": rsv})
    return in_maps


def _get_runner(loop_iters=None):
    """Persistent PJRT runner (mirror of bass2jax.run_bass_via_pjrt, but the
    jitted callable is cached so repeated calls skip re-trace/re-compile)."""
    key = ("runner", loop_iters)
    if key in _cache:
        return _cache[key]
    import concourse.mybir as mybir
    from concourse import bass2jax
    import jax
    from jax.sharding import Mesh, PartitionSpec
    from jax.experimental.shard_map import shard_map

    nc = _build_program(loop_iters)
    bass2jax.install_neuronx_cc_hook()

    partition_name = (nc.partition_id_tensor.name
                      if nc.partition_id_tensor else None)
    in_names, out_names, out_avals = [], [], []
    for alloc in nc.m.functions[0].allocations:
        if not isinstance(alloc, mybir.MemoryLocationSet):
            continue
        name = alloc.memorylocations[0].name
        if alloc.kind == "ExternalInput":
            if name != partition_name:
                in_names.append(name)
        elif alloc.kind == "ExternalOutput":
            out_names.append(name)
            out_avals.append(jax.core.ShapedArray(
                tuple(alloc.tensor_shape), mybir.dt.np(alloc.dtype)))
    n_params = len(in_names)
    all_names = in_names + out_names
    if partition_name is not None:
        all_names = all_names + [partition_name]

    def _body(*args):
        operands = list(args)
        if partition_name is not None:
            operands.append(bass2jax.partition_id_tensor())
        outs = bass2jax._bass_exec_p.bind(
            *operands,
            out_avals=tuple(out_avals),
            in_names=tuple(all_names),
            out_names=tuple(out_names),
            lowering_input_output_aliases=(),
            sim_require_finite=True,
            sim_require_nnan=True,
            nc=nc,
        )
        return tuple(outs)

    devices = jax.devices()[:NCORES]
    mesh = Mesh(np.asarray(devices), ("core",))
    n_outs = len(out_names)
    sharded = jax.jit(
        shard_map(_body, mesh=mesh,
                  in_specs=(PartitionSpec("core"),) * (n_params + n_outs),
                  out_specs=(PartitionSpec("core"),) * n_outs,
                  check_rep=False),
        keep_unused=True,
    )
    zero_outs = [np.zeros((NCORES * a.shape[0], *a.shape[1:]), a.dtype)
                 for a in out_avals]
    runner = {"fn": sharded, "in_names": in_names, "out_names": out_names,
              "out_avals": out_avals, "zero_outs": zero_outs}
    _cache[key] = runner
    return runner


class _Res:
    def __init__(self, results):
        self.results = results
        self.exec_time_ns = None
        self.instructions_and_trace = None


def _run_device(in_maps, trace=False):
    import jax
    r = _get_runner()
    concat_in = [np.concatenate([m[name] for m in in_maps], axis=0)
                 for name in r["in_names"]]
    out_arrs = r["fn"](*concat_in, *r["zero_outs"])
    jax.block_until_ready(out_arrs)
    results = [
        {name: np.asarray(out_arrs[i]).reshape(NCORES, *r["out_avals"][i].shape)[c]
         for i, name in enumerate(r["out_names"])}
        for c in range(NCORES)
    ]
    return _Res(results)


def _time_variant(in_maps, loop_iters, n):
    import time
    import jax
    r = _get_runner(loop_iters)
    concat_in = [np.concatenate([m[name] for m in in_maps], axis=0)
                 for name in r["in_names"]]
    dev_in = [jax.device_put(x) for x in concat_in]
    dev_zero = [jax.device_put(z) for z in r["zero_outs"]]
    jax.block_until_ready(dev_in + dev_zero)
    jax.block_until_ready(r["fn"](*dev_in, *dev_zero))  # warmup
    times = []
    for _ in range(n):
        t0 = time.perf_counter()
        jax.block_until_ready(r["fn"](*dev_in, *dev_zero))
        times.append(time.perf_counter() - t0)
    return times


def _time_runs(in_maps, n=6, iters=1024):
    """Estimate per-iteration HW time via slope between a 1-iter and a
    (1+iters)-iter device-side loop around the whole kernel body.
    Dispatch overhead cancels in the difference."""
    t1 = _time_variant(in_maps, 1, n)
    tk = _time_variant(in_maps, 1 + iters, n)
    per_iter = (min(tk) - min(t1)) / iters
    return per_iter, t1, tk


def kernel(**inputs):
    return _kernel_impl(inputs, trace=False)[0]


def _kernel_impl(inputs, trace=False):
    pp = np.asarray(inputs["predicted_points"], np.float32)
    ps_ = np.asarray(inputs["predicted_sdfs"], np.float32)
    pc = np.asarray(inputs["predicted_colors"], np.float32)
    rp = np.asarray(inputs["ref_points"], np.float32)
    rs_ = np.asarray(inputs["ref_sdfs"], np.float32)
    rc = np.asarray(inputs["ref_colors"], np.float32)

    pn2 = (pp * pp).sum(-1)  # [B, N] f32
    rm2 = (rp * rp).sum(-1)  # [B, M]

    in_maps = _make_in_maps(pp, rp, pn2, rm2)
    res = _run_device(in_maps, trace=trace)
    outs = res.results

    colmin = np.empty((B, M), np.float32)
    closest = np.empty((B, M), np.int64)
    rowmin = np.full((B, N), np.inf, np.float32)

    ar32 = np.arange(WIN)
    mloc = np.arange(MC)
    for cid in range(NCORES):
        b, c = cid // CHUNKS, cid % CHUNKS
        sl = slice(c * MC, (c + 1) * MC)
        winm = np.asarray(outs[cid]["winm"])        # [128, NB*wout]
        rowacc = np.asarray(outs[cid]["rowacc"])    # [128, N]
        # m = c*MC + beta*128 + p  ->  [MC, NWIN]
        if RED_MODE == "f16":
            wm = (winm.reshape(128, NB, NWIN, 2).min(axis=3)
                  .transpose(1, 0, 2).reshape(MC, NWIN))
        else:
            wm = winm.reshape(128, NB, NWIN).transpose(1, 0, 2).reshape(MC, NWIN)
        if RED_MODE == "f16":
            # device winmins are fp16-rounded: re-examine the top-2 windows
            # exactly so near-ties cannot flip the argmin
            top2 = np.argpartition(wm, 1, axis=1)[:, :2]
            top2 = np.sort(top2, axis=1)            # keep n ascending
            n0s = top2 * WIN                        # [MC, 2]
            nidx = (n0s[:, :, None] + ar32[None, None, :]).reshape(MC, 2 * WIN)
        else:
            wstar = np.argmin(wm, axis=1)           # [MC]
            nidx = wstar[:, None] * WIN + ar32[None, :]
        pw = pp[b][nidx]                            # [MC, W, 3]
        rv = rp[b, sl]                              # [MC, 3]
        d2w = (pn2[b][nidx] + rm2[b, sl][:, None]
               - 2.0 * np.einsum('mwd,md->mw', pw, rv, dtype=np.float32))
        d2w = np.maximum(d2w, 0.0)
        j = np.argmin(d2w, axis=1)
        colmin[b, sl] = d2w[mloc, j]
        closest[b, sl] = nidx[mloc, j]
        rowmin[b] = np.minimum(rowmin[b],
                               rowacc.min(axis=0).astype(np.float32))

    rowmin = np.maximum(rowmin, 0.0)

    cham_xy = rowmin.mean(axis=1)    # [B]  pred -> ref
    cham_yx = colmin.mean(axis=1)    # [B]  ref -> pred
    chamfer = np.float32((cham_xy + cham_yx).mean())

    bi = np.arange(B)[:, None]
    g_sdfs = rs_[bi, closest, :]     # [B, M, 1]
    sdf_l1 = np.float32(np.abs(g_sdfs - ps_).mean())
    g_cols = rc[bi, closest, :]      # [B, M, 3]
    color_l1 = np.float32(np.abs(g_cols - pc).mean())

    out = np.stack([sdf_l1, color_l1, chamfer]).astype(np.float32)
    return out, res

